# revision 2
# baseline (speedup 1.0000x reference)
"""MoE layer (top-2 routing) on 8 Trainium2 NeuronCores.

Strategy (expert-parallel, per the sharding hint):
  - Router (logits -> softmax -> top-2 -> combine weights, aux loss) is
    computed on host with the exact same eager jax ops as the reference,
    so routing decisions match the reference bitwise.
  - Tokens are dispatched (host-side all-to-all) to 8 expert shards: core e
    receives the tokens whose top-2 set contains expert e, padded to a
    common capacity C.
  - Each core runs the expert FFN  y = relu(x @ W1[e] + b1[e]) @ W2[e] + b2[e]
    as a Bass/Tile kernel: tokens live in the matmul free dimension
    (activations stay transposed, [D, C]), so no on-device transposes are
    needed anywhere.  Matmuls run in float32r (TF32-like, ~1.5e-4 rel err,
    4x the fp32 matmul rate), accumulation in fp32 PSUM.
  - Host combines: out[t] = g_a * y_ea[t] + g_b * y_eb[t]  (expert-index
    order, matching the reference's accumulation order).
"""

import numpy as np

B, S, D, F, E = 8, 2048, 512, 2048, 8
P = 128
NCH = 512                 # tokens per chunk == PSUM bank free dim (fp32)
KD, MF = D // P, F // P   # 4, 16  (mm1: K-tiles over D, M-tiles over F)
KF, MD = F // P, D // P   # 16, 4  (mm2: K-tiles over F, M-tiles over D)

_PROGRAM_CACHE = {}


def build_ffn_program(C, reps=1, dt_name="float32r"):
    """Per-core expert-FFN program: yT[D,C] = FFN(xT[D,C]) with weights resident."""
    import concourse.bacc as bacc
    import concourse.tile as tile
    from concourse import mybir

    DT = getattr(mybir.dt, dt_name)
    f32 = mybir.dt.float32
    AF = mybir.ActivationFunctionType
    assert C % NCH == 0

    nc = bacc.Bacc(None, target_bir_lowering=False, debug=False)
    xT = nc.dram_tensor("xT", [KD, P, C], DT, kind="ExternalInput")
    w1 = nc.dram_tensor("w1", [KD, P, F], DT, kind="ExternalInput")
    w2 = nc.dram_tensor("w2", [KF, P, D], DT, kind="ExternalInput")
    b1 = nc.dram_tensor("b1", [P, MF], f32, kind="ExternalInput")
    b2 = nc.dram_tensor("b2", [P, MD], f32, kind="ExternalInput")
    yT = nc.dram_tensor("yT", [MD, P, C], f32, kind="ExternalOutput")

    with tile.TileContext(nc) as tc:
        with (
            tc.tile_pool(name="wpool", bufs=1) as wpool,
            tc.tile_pool(name="xpool", bufs=3) as xpool,
            tc.tile_pool(name="hpool", bufs=2) as hpool,
            tc.tile_pool(name="ypool", bufs=3) as ypool,
            tc.tile_pool(name="ps1", bufs=2, space="PSUM") as ps1,
            tc.tile_pool(name="ps2", bufs=2, space="PSUM") as ps2,
        ):
            w1t = wpool.tile([P, KD, F], DT, tag="w1")
            w2t = wpool.tile([P, KF, D], DT, tag="w2")
            b1t = wpool.tile([P, MF], f32, tag="b1")
            b2t = wpool.tile([P, MD], f32, tag="b2")
            for k in range(KD):
                nc.sync.dma_start(w1t[:, k, :], w1[k])
            for k in range(KF):
                nc.sync.dma_start(w2t[:, k, :], w2[k])
            nc.sync.dma_start(b1t[:], b1[:])
            nc.sync.dma_start(b2t[:], b2[:])

            def body():
                for c in range(C // NCH):
                    sl = slice(c * NCH, (c + 1) * NCH)
                    xt = xpool.tile([P, KD, NCH], DT, tag="x")
                    for k in range(KD):
                        nc.sync.dma_start(xt[:, k, :], xT[k, :, sl])
                    ht = hpool.tile([P, KF, NCH], DT, tag="h")
                    for m in range(MF):
                        ps = ps1.tile([P, NCH], f32, tag="ps1")
                        for k in range(KD):
                            nc.tensor.matmul(
                                ps[:],
                                lhsT=w1t[:, k, m * P:(m + 1) * P],
                                rhs=xt[:, k, :],
                                start=(k == 0),
                                stop=(k == KD - 1),
                            )
                        nc.scalar.activation(ht[:, m, :], ps[:], AF.Relu,
                                             bias=b1t[:, m:m + 1])
                    yt = ypool.tile([P, MD, NCH], f32, tag="y")
                    for m in range(MD):
                        ps = ps2.tile([P, NCH], f32, tag="ps2")
                        for k in range(KF):
                            nc.tensor.matmul(
                                ps[:],
                                lhsT=w2t[:, k, m * P:(m + 1) * P],
                                rhs=ht[:, k, :],
                                start=(k == 0),
                                stop=(k == KF - 1),
                            )
                        nc.scalar.activation(yt[:, m, :], ps[:], AF.Identity,
                                             bias=b2t[:, m:m + 1])
                        nc.sync.dma_start(yT[m, :, sl], yt[:, m, :])

            if reps == 1:
                body()
            else:
                with tc.For_i(0, reps):
                    body()

    nc.finalize()
    return nc


def _get_program(C, reps=1, dt_name="float32r"):
    key = (C, reps, dt_name)
    if key not in _PROGRAM_CACHE:
        _PROGRAM_CACHE[key] = build_ffn_program(C, reps, dt_name)
    return _PROGRAM_CACHE[key]


def route_host(x, Wr, br):
    """Router computed with the reference's exact eager jax ops (bitwise match)."""
    import jax
    import jax.numpy as jnp

    logits = jnp.einsum('bsd,de->bse', jnp.asarray(x), jnp.asarray(Wr)) \
        + jnp.asarray(br)
    gate = jax.nn.softmax(logits, axis=-1)
    top2_val, top2_idx = jax.lax.top_k(gate, 2)
    expert_prob = gate.mean(axis=(0, 1))
    aux_loss = jnp.sum(expert_prob * jnp.log(expert_prob + 1e-9))
    return (np.asarray(top2_val), np.asarray(top2_idx),
            np.float32(np.asarray(aux_loss)))


def make_dispatch(t2i):
    """Token lists / slots per expert from the [T,2] top-2 index array."""
    T = t2i.shape[0]
    e1, e2 = t2i[:, 0], t2i[:, 1]
    toks = [np.nonzero((e1 == e) | (e2 == e))[0] for e in range(E)]
    counts = np.array([len(t) for t in toks])
    slot = np.zeros((T, 2), np.int64)
    for e in range(E):
        p_of = np.empty(T, np.int64)
        p_of[toks[e]] = np.arange(len(toks[e]))
        for r in range(2):
            m = t2i[:, r] == e
            slot[m, r] = p_of[m]
    return toks, counts, slot


def kernel(x, Wr, br, W1, b1, W2, b2, _reps=1, _dt_name="float32r",
           _return_results=False):
    from concourse.bass_utils import run_bass_kernel_spmd

    x = np.asarray(x, np.float32)
    Wr = np.asarray(Wr, np.float32)
    br = np.asarray(br, np.float32)
    W1 = np.asarray(W1, np.float32)
    b1 = np.asarray(b1, np.float32)
    W2 = np.asarray(W2, np.float32)
    b2 = np.asarray(b2, np.float32)

    T = B * S
    x_flat = x.reshape(T, D)

    top2_val, top2_idx, aux_loss = route_host(x, Wr, br)
    t2i = top2_idx.reshape(T, 2)
    t2v = top2_val.reshape(T, 2)

    toks, counts, slot = make_dispatch(t2i)
    C = max(NCH, int(-(-counts.max() // NCH) * NCH))

    in_maps = []
    for e in range(E):
        xe = np.zeros((C, D), np.float32)
        xe[:counts[e]] = x_flat[toks[e]]
        in_maps.append({
            "xT": np.ascontiguousarray(xe.T).reshape(KD, P, C),
            "w1": np.ascontiguousarray(W1[e]).reshape(KD, P, F),
            "w2": np.ascontiguousarray(W2[e]).reshape(KF, P, D),
            "b1": np.ascontiguousarray(b1[e].reshape(MF, P).T),
            "b2": np.ascontiguousarray(b2[e].reshape(MD, P).T),
        })

    nc = _get_program(C, _reps, _dt_name)
    res = run_bass_kernel_spmd(nc, in_maps, list(range(E)), trace=False)

    # y_stack[e, c, :] = expert e's output for its c-th assigned token
    y_stack = np.stack([res.results[e]["yT"].reshape(D, C).T for e in range(E)])

    # Combine in expert-index order (matches the reference's e-loop order)
    e1, e2 = t2i[:, 0], t2i[:, 1]
    r_first = np.where(e1 < e2, 0, 1)
    ar = np.arange(T)
    ga = t2v[ar, r_first]
    gb = t2v[ar, 1 - r_first]
    ea = t2i[ar, r_first]
    eb = t2i[ar, 1 - r_first]
    sa = slot[ar, r_first]
    sb = slot[ar, 1 - r_first]
    out_flat = ga[:, None] * y_stack[ea, sa] + gb[:, None] * y_stack[eb, sb]
    out = out_flat.reshape(B, S, D).astype(np.float32)

    if _return_results:
        return out, aux_loss, res
    return out, aux_loss


# revision 6
# speedup vs baseline: 1.0339x; 1.0339x over previous
"""MoE layer (top-2 routing) on 8 Trainium2 NeuronCores.

Strategy (expert-parallel, per the sharding hint):
  - Router (logits -> softmax -> top-2 -> combine weights, aux loss) is
    computed on host with the exact same eager jax ops as the reference,
    so routing decisions match the reference bitwise.
  - Tokens are dispatched (host-side all-to-all) to 8 expert shards: core e
    receives the tokens whose top-2 set contains expert e, padded to a
    common capacity C.
  - Each core runs the expert FFN  y = relu(x @ W1[e] + b1[e]) @ W2[e] + b2[e]
    as a Bass/Tile kernel: tokens live in the matmul free dimension
    (activations stay transposed, [D, C]), so no on-device transposes are
    needed anywhere.  Matmuls run in float32r (TF32-like, ~1.5e-4 rel err,
    4x the fp32 matmul rate), accumulation in fp32 PSUM.
  - Host combines: out[t] = g_a * y_ea[t] + g_b * y_eb[t]  (expert-index
    order, matching the reference's accumulation order).
"""

import numpy as np

B, S, D, F, E = 8, 2048, 512, 2048, 8
P = 128
NCH = 512                 # tokens per chunk == PSUM bank free dim (fp32)
KD, MF = D // P, F // P   # 4, 16  (mm1: K-tiles over D, M-tiles over F)
KF, MD = F // P, D // P   # 16, 4  (mm2: K-tiles over F, M-tiles over D)

_PROGRAM_CACHE = {}


def build_ffn_program(C, reps=1, dt_name="float32r"):
    """Per-core expert-FFN program: yT[D,C] = FFN(xT[D,C]) with weights resident."""
    import concourse.bacc as bacc
    import concourse.tile as tile
    from concourse import mybir

    DT = getattr(mybir.dt, dt_name)
    f32 = mybir.dt.float32
    AF = mybir.ActivationFunctionType
    assert C % NCH == 0

    nc = bacc.Bacc(None, target_bir_lowering=False, debug=False)
    xT = nc.dram_tensor("xT", [KD, P, C], DT, kind="ExternalInput")
    w1 = nc.dram_tensor("w1", [KD, P, F], DT, kind="ExternalInput")
    w2 = nc.dram_tensor("w2", [KF, P, D], DT, kind="ExternalInput")
    b1 = nc.dram_tensor("b1", [P, MF], f32, kind="ExternalInput")
    b2 = nc.dram_tensor("b2", [P, MD], f32, kind="ExternalInput")
    yT = nc.dram_tensor("yT", [MD, P, C], f32, kind="ExternalOutput")

    with tile.TileContext(nc) as tc:
        with (
            tc.tile_pool(name="wpool", bufs=1) as wpool,
            tc.tile_pool(name="xpool", bufs=3) as xpool,
            tc.tile_pool(name="hpool", bufs=2) as hpool,
            tc.tile_pool(name="ypool", bufs=3) as ypool,
            tc.tile_pool(name="ps1", bufs=2, space="PSUM") as ps1,
            tc.tile_pool(name="ps2", bufs=2, space="PSUM") as ps2,
        ):
            w1t = wpool.tile([P, KD, F], DT, tag="w1")
            w2t = wpool.tile([P, KF, D], DT, tag="w2")
            b1t = wpool.tile([P, MF], f32, tag="b1")
            b2t = wpool.tile([P, MD], f32, tag="b2")
            # Issue order matters: the HWDGE queue drains in FIFO order, so
            # load w1 (needed by chunk 0's mm1) first; w2 (only needed when
            # chunk 0 reaches mm2) is issued after chunk 0's x DMAs by the
            # callback below.
            for k in range(KD):
                nc.sync.dma_start(w1t[:, k, :], w1[k])
            nc.sync.dma_start(b1t[:], b1[:])

            def load_w2():
                for k in range(KF):
                    nc.sync.dma_start(w2t[:, k, :], w2[k])
                nc.sync.dma_start(b2t[:], b2[:])

            def body(first=False):
                for c in range(C // NCH):
                    sl = slice(c * NCH, (c + 1) * NCH)
                    xt = xpool.tile([P, KD, NCH], DT, tag="x")
                    for k in range(KD):
                        nc.sync.dma_start(xt[:, k, :], xT[k, :, sl])
                    if first and c == 0:
                        load_w2()
                    ht = hpool.tile([P, KF, NCH], DT, tag="h")
                    for m in range(MF):
                        ps = ps1.tile([P, NCH], f32, tag="ps1")
                        for k in range(KD):
                            nc.tensor.matmul(
                                ps[:],
                                lhsT=w1t[:, k, m * P:(m + 1) * P],
                                rhs=xt[:, k, :],
                                start=(k == 0),
                                stop=(k == KD - 1),
                            )
                        nc.scalar.activation(ht[:, m, :], ps[:], AF.Relu,
                                             bias=b1t[:, m:m + 1])
                    yt = ypool.tile([P, MD, NCH], f32, tag="y")
                    for m in range(MD):
                        ps = ps2.tile([P, NCH], f32, tag="ps2")
                        for k in range(KF):
                            nc.tensor.matmul(
                                ps[:],
                                lhsT=w2t[:, k, m * P:(m + 1) * P],
                                rhs=ht[:, k, :],
                                start=(k == 0),
                                stop=(k == KF - 1),
                            )
                        nc.scalar.activation(yt[:, m, :], ps[:], AF.Identity,
                                             bias=b2t[:, m:m + 1])
                        nc.sync.dma_start(yT[m, :, sl], yt[:, m, :])

            if reps == 1:
                body(first=True)
            else:
                body(first=True)
                with tc.For_i(0, reps - 1):
                    body()

    nc.finalize()
    return nc


def _get_program(C, reps=1, dt_name="float32r"):
    key = (C, reps, dt_name)
    if key not in _PROGRAM_CACHE:
        _PROGRAM_CACHE[key] = build_ffn_program(C, reps, dt_name)
    return _PROGRAM_CACHE[key]


def route_host(x, Wr, br):
    """Router computed with the reference's exact eager jax ops (bitwise match)."""
    import jax
    import jax.numpy as jnp

    logits = jnp.einsum('bsd,de->bse', jnp.asarray(x), jnp.asarray(Wr)) \
        + jnp.asarray(br)
    gate = jax.nn.softmax(logits, axis=-1)
    top2_val, top2_idx = jax.lax.top_k(gate, 2)
    expert_prob = gate.mean(axis=(0, 1))
    aux_loss = jnp.sum(expert_prob * jnp.log(expert_prob + 1e-9))
    return (np.asarray(top2_val), np.asarray(top2_idx),
            np.float32(np.asarray(aux_loss)))


def make_dispatch(t2i):
    """Token lists / slots per expert from the [T,2] top-2 index array."""
    T = t2i.shape[0]
    e1, e2 = t2i[:, 0], t2i[:, 1]
    toks = [np.nonzero((e1 == e) | (e2 == e))[0] for e in range(E)]
    counts = np.array([len(t) for t in toks])
    slot = np.zeros((T, 2), np.int64)
    for e in range(E):
        p_of = np.empty(T, np.int64)
        p_of[toks[e]] = np.arange(len(toks[e]))
        for r in range(2):
            m = t2i[:, r] == e
            slot[m, r] = p_of[m]
    return toks, counts, slot


def kernel(x, Wr, br, W1, b1, W2, b2, _reps=1, _dt_name="float32r",
           _return_results=False):
    from concourse.bass_utils import run_bass_kernel_spmd

    x = np.asarray(x, np.float32)
    Wr = np.asarray(Wr, np.float32)
    br = np.asarray(br, np.float32)
    W1 = np.asarray(W1, np.float32)
    b1 = np.asarray(b1, np.float32)
    W2 = np.asarray(W2, np.float32)
    b2 = np.asarray(b2, np.float32)

    T = B * S
    x_flat = x.reshape(T, D)

    top2_val, top2_idx, aux_loss = route_host(x, Wr, br)
    t2i = top2_idx.reshape(T, 2)
    t2v = top2_val.reshape(T, 2)

    toks, counts, slot = make_dispatch(t2i)
    C = max(NCH, int(-(-counts.max() // NCH) * NCH))

    in_maps = []
    for e in range(E):
        xe = np.zeros((C, D), np.float32)
        xe[:counts[e]] = x_flat[toks[e]]
        in_maps.append({
            "xT": np.ascontiguousarray(xe.T).reshape(KD, P, C),
            "w1": np.ascontiguousarray(W1[e]).reshape(KD, P, F),
            "w2": np.ascontiguousarray(W2[e]).reshape(KF, P, D),
            "b1": np.ascontiguousarray(b1[e].reshape(MF, P).T),
            "b2": np.ascontiguousarray(b2[e].reshape(MD, P).T),
        })

    nc = _get_program(C, _reps, _dt_name)
    res = run_bass_kernel_spmd(nc, in_maps, list(range(E)), trace=False)

    # y_stack[e, c, :] = expert e's output for its c-th assigned token
    y_stack = np.stack([res.results[e]["yT"].reshape(D, C).T for e in range(E)])

    # Combine in expert-index order (matches the reference's e-loop order)
    e1, e2 = t2i[:, 0], t2i[:, 1]
    r_first = np.where(e1 < e2, 0, 1)
    ar = np.arange(T)
    ga = t2v[ar, r_first]
    gb = t2v[ar, 1 - r_first]
    ea = t2i[ar, r_first]
    eb = t2i[ar, 1 - r_first]
    sa = slot[ar, r_first]
    sb = slot[ar, 1 - r_first]
    out_flat = ga[:, None] * y_stack[ea, sa] + gb[:, None] * y_stack[eb, sb]
    out = out_flat.reshape(B, S, D).astype(np.float32)

    if _return_results:
        return out, aux_loss, res
    return out, aux_loss


# revision 7
# speedup vs baseline: 1.4593x; 1.4114x over previous
"""MoE layer (top-2 routing) on 8 Trainium2 NeuronCores.

Strategy (expert-parallel, per the sharding hint):
  - Router (logits -> softmax -> top-2 -> combine weights, aux loss) is
    computed on host with the exact same eager jax ops as the reference,
    so routing decisions match the reference bitwise.
  - Tokens are dispatched (host-side all-to-all) to 8 expert shards: core e
    receives the tokens whose top-2 set contains expert e, padded to a
    common capacity C.
  - Each core runs the expert FFN  y = relu(x @ W1[e] + b1[e]) @ W2[e] + b2[e]
    as a Bass/Tile kernel: tokens live in the matmul free dimension
    (activations stay transposed, [D, C]), so no on-device transposes are
    needed anywhere.  Matmuls run in float32r (TF32-like, ~1.5e-4 rel err,
    4x the fp32 matmul rate), accumulation in fp32 PSUM.
  - Host combines: out[t] = g_a * y_ea[t] + g_b * y_eb[t]  (expert-index
    order, matching the reference's accumulation order).
"""

import numpy as np

B, S, D, F, E = 8, 2048, 512, 2048, 8
P = 128
NCH = 512                 # tokens per chunk == PSUM bank free dim (fp32)
KD, MF = D // P, F // P   # 4, 16  (mm1: K-tiles over D, M-tiles over F)
KF, MD = F // P, D // P   # 16, 4  (mm2: K-tiles over F, M-tiles over D)

_PROGRAM_CACHE = {}


def build_ffn_program(C, reps=1, dt_name="float32r"):
    """Per-core expert-FFN program: yT[D,C] = FFN(xT[D,C]) with weights resident."""
    import concourse.bacc as bacc
    import concourse.tile as tile
    from concourse import mybir

    DT = getattr(mybir.dt, dt_name)
    f32 = mybir.dt.float32
    AF = mybir.ActivationFunctionType
    assert C % NCH == 0

    nc = bacc.Bacc(None, target_bir_lowering=False, debug=False)
    xT = nc.dram_tensor("xT", [KD, P, C], DT, kind="ExternalInput")
    w1 = nc.dram_tensor("w1", [KD, P, F], DT, kind="ExternalInput")
    w2 = nc.dram_tensor("w2", [KF, P, D], DT, kind="ExternalInput")
    b1 = nc.dram_tensor("b1", [P, MF], f32, kind="ExternalInput")
    b2 = nc.dram_tensor("b2", [P, MD], f32, kind="ExternalInput")
    yT = nc.dram_tensor("yT", [MD, P, C], f32, kind="ExternalOutput")

    with tile.TileContext(nc) as tc:
        with (
            tc.tile_pool(name="wpool", bufs=1) as wpool,
            tc.tile_pool(name="xpool", bufs=3) as xpool,
            tc.tile_pool(name="hpool", bufs=2) as hpool,
            tc.tile_pool(name="ypool", bufs=3) as ypool,
            tc.tile_pool(name="ps1", bufs=4, space="PSUM") as ps1,
            tc.tile_pool(name="ps2", bufs=4, space="PSUM") as ps2,
        ):
            w1t = wpool.tile([P, KD, F], DT, tag="w1")
            w2t = wpool.tile([P, KF, D], DT, tag="w2")
            b1t = wpool.tile([P, MF], f32, tag="b1")
            b2t = wpool.tile([P, MD], f32, tag="b2")
            # Issue order matters: the HWDGE queue drains in FIFO order, so
            # load w1 (needed by chunk 0's mm1) first; w2 (only needed when
            # chunk 0 reaches mm2) is issued after chunk 0's x DMAs by the
            # callback below.
            for k in range(KD):
                nc.sync.dma_start(w1t[:, k, :], w1[k])
            nc.sync.dma_start(b1t[:], b1[:])

            def load_w2():
                for k in range(KF):
                    nc.sync.dma_start(w2t[:, k, :], w2[k])
                nc.sync.dma_start(b2t[:], b2[:])

            def body(first=False):
                for c in range(C // NCH):
                    sl = slice(c * NCH, (c + 1) * NCH)
                    xt = xpool.tile([P, KD, NCH], DT, tag="x")
                    for k in range(KD):
                        nc.sync.dma_start(xt[:, k, :], xT[k, :, sl])
                    if first and c == 0:
                        load_w2()
                    ht = hpool.tile([P, KF, NCH], DT, tag="h")
                    for m in range(MF):
                        ps = ps1.tile([P, NCH], f32, tag="ps1")
                        for k in range(KD):
                            nc.tensor.matmul(
                                ps[:],
                                lhsT=w1t[:, k, m * P:(m + 1) * P],
                                rhs=xt[:, k, :],
                                start=(k == 0),
                                stop=(k == KD - 1),
                            )
                        nc.scalar.activation(ht[:, m, :], ps[:], AF.Relu,
                                             bias=b1t[:, m:m + 1])
                    yt = ypool.tile([P, MD, NCH], f32, tag="y")
                    for m in range(MD):
                        ps = ps2.tile([P, NCH], f32, tag="ps2")
                        for k in range(KF):
                            nc.tensor.matmul(
                                ps[:],
                                lhsT=w2t[:, k, m * P:(m + 1) * P],
                                rhs=ht[:, k, :],
                                start=(k == 0),
                                stop=(k == KF - 1),
                            )
                        nc.scalar.activation(yt[:, m, :], ps[:], AF.Identity,
                                             bias=b2t[:, m:m + 1])
                        nc.sync.dma_start(yT[m, :, sl], yt[:, m, :])

            if reps == 1:
                body(first=True)
            else:
                body(first=True)
                with tc.For_i(0, reps - 1):
                    body()

    nc.finalize()
    return nc


def _get_program(C, reps=1, dt_name="float32r"):
    key = (C, reps, dt_name)
    if key not in _PROGRAM_CACHE:
        _PROGRAM_CACHE[key] = build_ffn_program(C, reps, dt_name)
    return _PROGRAM_CACHE[key]


def route_host(x, Wr, br):
    """Router computed with the reference's exact eager jax ops (bitwise match)."""
    import jax
    import jax.numpy as jnp

    logits = jnp.einsum('bsd,de->bse', jnp.asarray(x), jnp.asarray(Wr)) \
        + jnp.asarray(br)
    gate = jax.nn.softmax(logits, axis=-1)
    top2_val, top2_idx = jax.lax.top_k(gate, 2)
    expert_prob = gate.mean(axis=(0, 1))
    aux_loss = jnp.sum(expert_prob * jnp.log(expert_prob + 1e-9))
    return (np.asarray(top2_val), np.asarray(top2_idx),
            np.float32(np.asarray(aux_loss)))


def make_dispatch(t2i):
    """Token lists / slots per expert from the [T,2] top-2 index array."""
    T = t2i.shape[0]
    e1, e2 = t2i[:, 0], t2i[:, 1]
    toks = [np.nonzero((e1 == e) | (e2 == e))[0] for e in range(E)]
    counts = np.array([len(t) for t in toks])
    slot = np.zeros((T, 2), np.int64)
    for e in range(E):
        p_of = np.empty(T, np.int64)
        p_of[toks[e]] = np.arange(len(toks[e]))
        for r in range(2):
            m = t2i[:, r] == e
            slot[m, r] = p_of[m]
    return toks, counts, slot


def kernel(x, Wr, br, W1, b1, W2, b2, _reps=1, _dt_name="float32r",
           _return_results=False):
    from concourse.bass_utils import run_bass_kernel_spmd

    x = np.asarray(x, np.float32)
    Wr = np.asarray(Wr, np.float32)
    br = np.asarray(br, np.float32)
    W1 = np.asarray(W1, np.float32)
    b1 = np.asarray(b1, np.float32)
    W2 = np.asarray(W2, np.float32)
    b2 = np.asarray(b2, np.float32)

    T = B * S
    x_flat = x.reshape(T, D)

    top2_val, top2_idx, aux_loss = route_host(x, Wr, br)
    t2i = top2_idx.reshape(T, 2)
    t2v = top2_val.reshape(T, 2)

    toks, counts, slot = make_dispatch(t2i)
    C = max(NCH, int(-(-counts.max() // NCH) * NCH))

    in_maps = []
    for e in range(E):
        xe = np.zeros((C, D), np.float32)
        xe[:counts[e]] = x_flat[toks[e]]
        in_maps.append({
            "xT": np.ascontiguousarray(xe.T).reshape(KD, P, C),
            "w1": np.ascontiguousarray(W1[e]).reshape(KD, P, F),
            "w2": np.ascontiguousarray(W2[e]).reshape(KF, P, D),
            "b1": np.ascontiguousarray(b1[e].reshape(MF, P).T),
            "b2": np.ascontiguousarray(b2[e].reshape(MD, P).T),
        })

    nc = _get_program(C, _reps, _dt_name)
    res = run_bass_kernel_spmd(nc, in_maps, list(range(E)), trace=False)

    # y_stack[e, c, :] = expert e's output for its c-th assigned token
    y_stack = np.stack([res.results[e]["yT"].reshape(D, C).T for e in range(E)])

    # Combine in expert-index order (matches the reference's e-loop order)
    e1, e2 = t2i[:, 0], t2i[:, 1]
    r_first = np.where(e1 < e2, 0, 1)
    ar = np.arange(T)
    ga = t2v[ar, r_first]
    gb = t2v[ar, 1 - r_first]
    ea = t2i[ar, r_first]
    eb = t2i[ar, 1 - r_first]
    sa = slot[ar, r_first]
    sb = slot[ar, 1 - r_first]
    out_flat = ga[:, None] * y_stack[ea, sa] + gb[:, None] * y_stack[eb, sb]
    out = out_flat.reshape(B, S, D).astype(np.float32)

    if _return_results:
        return out, aux_loss, res
    return out, aux_loss
